# revision 35
# baseline (speedup 1.0000x reference)
"""CenterLoss kernel for Trainium2 — v8: 6 SWDGE chunks + 2 PE-window chunks.

loss = mean_i( clip( ||x_i - centers[labels[i]]||^2, 1e-12, 1e12 ) )

The per-sample center-row fetch is the serial bottleneck: each 128-row SWDGE
indirect call costs ~1.41us of GPSIMD ucode (994ns fixed; ring holds only
128 descriptors). v8 keeps 6 chunks on SWDGE (8.5us chain instead of
11.3us) and moves the last 2 chunks to a one-hot matmul on the idle PE:

  The host sorts samples by label (pure input permutation — the mean is
  order-invariant; per-core ordering is the sharding strategy). Each
  128-sample chunk's sorted labels span < 192 consecutive classes (max 145
  for the input spec, 172 worst random trial), so the host ships one
  contiguous 192-row center window per PE chunk (slicing only) plus
  rr = label - window_base. On-device: rr broadcast across partitions via
  a K=1 ones-matmul into PSUM; one-hot halves by DVE is_equal against a
  per-partition iota (u=0: K=128 rows, u=1: K=64 rows); exact row select
  via P^T.T @ W accumulated in fp32 PSUM. Window DMAs ride the Scalar
  HWDGE queue (+384KB only, vs +1MB in a 4+4 split that was DMA-bound).
  Window compute sits FIRST in program order so it fills the idle window
  while SWDGE descriptor-gen runs, keeping the tail = SWDGE chunk 5.

  idx rides a pre-TileContext DMA (manual semaphore, wait attached to the
  first gather after tile scheduling — an in-block wait deadlocks the tile
  scheduler's block-local sim).

Numerics: x/centers bf16 on the wire, fp32 accumulation: ~1e-5 relative
error on the mean vs the fp32 reference (gate 2e-2). _prep_core returns
None (caller raises) if any window chunk spans >= 192 classes.
"""

import sys

import numpy as np

if "/opt/trn_rl_repo" not in sys.path:
    sys.path.insert(0, "/opt/trn_rl_repo")

import ml_dtypes

_B, _D, _C = 8192, 512, 8000
_N_CORES = 8
_B_LOC = _B // _N_CORES  # 1024 rows per core
_P = 128
_M = _B_LOC // _P  # 8 chunks of 128 rows
_MS = 6  # chunks gathered via SWDGE (0.._MS)
_MW = _M - _MS  # chunks selected via PE windows (_MS.._M)
_W = 192  # center window rows per chunk (halves of 128 + 64)
_N_QUEUES = 2
_CLAMP_MIN, _CLAMP_MAX = 1e-12, 1e12

_cache: dict = {}


def _build():
    import concourse.bass as bass
    import concourse.tile as tile
    from concourse import bacc, mybir

    nc = bacc.Bacc(
        "TRN2",
        debug=False,
        enable_asserts=False,
        target_bir_lowering=False,
        num_devices=_N_CORES,
        num_swdge_queues=_N_QUEUES,
    )
    # x chunk-major: x_d[p, t*512:(t+1)*512] = x_sorted[t*128+p], bf16
    x_d = nc.dram_tensor("x", [_P, _M * _D], mybir.dt.bfloat16, kind="ExternalInput")
    # SWDGE offsets: idx[p, m] = label_sorted[m*128+p], int32, chunks 0-5
    lab_d = nc.dram_tensor("labels_packed", [_P, _MS], mybir.dt.int32, kind="ExternalInput")
    cen_d = nc.dram_tensor("centers", [_C, _D], mybir.dt.bfloat16, kind="ExternalInput")
    # window stacks for chunks 6-7: block 2k   = rows 0-127  of window k,
    #                               block 2k+1 = rows 128-191 (parts 0-63)
    w_d = nc.dram_tensor("wins", [_P, _MW * 2 * _D], mybir.dt.bfloat16, kind="ExternalInput")
    # rr[0, k*128+j] = label_sorted[(6+k)*128+j] - base_k (< 192, exact bf16)
    rr_d = nc.dram_tensor("rr", [1, _MW * _P], mybir.dt.bfloat16, kind="ExternalInput")
    out_d = nc.dram_tensor("out", [_P, _M], mybir.dt.float32, kind="ExternalOutput")

    # idx load before TileContext entry (skips tile entry drains).
    idx_sb = nc.alloc_sbuf_tensor("idx_early", [_P, _MS], mybir.dt.int32)
    idx_sem = nc.alloc_semaphore("idx_sem")
    nc.sync.dma_start(out=idx_sb[:], in_=lab_d.ap()).then_inc(idx_sem, 16)

    with tile.TileContext(nc) as tc:
        with (
            tc.tile_pool(name="big", bufs=1) as big,
            tc.tile_pool(name="work", bufs=4) as work,
            tc.tile_pool(name="misc", bufs=1) as misc,
            tc.tile_pool(name="psum_rr", bufs=1, space="PSUM") as psum_rr,
            tc.tile_pool(name="psum_g", bufs=2, space="PSUM") as psum_g,
        ):
            gather_h = []

            # window-path constants
            ones = misc.tile([1, _P], mybir.dt.bfloat16)
            nc.gpsimd.memset(ones[:], 1.0)
            iota_col = misc.tile([_P, 1], mybir.dt.float32)
            nc.gpsimd.iota(
                iota_col[:], pattern=[[0, 1]], base=0, channel_multiplier=1,
                allow_small_or_imprecise_dtypes=True,
            )

            # Sync queue: rr (1 descriptor) first, then x.
            rrsb = misc.tile([1, _MW * _P], mybir.dt.bfloat16)
            nc.sync.dma_start(out=rrsb[:], in_=rr_d.ap())
            xsb = big.tile([_P, _M * _D], mybir.dt.bfloat16)
            nc.sync.dma_start(out=xsb[:], in_=x_d.ap())
            # Scalar HWDGE queue: the 384KB window stack.
            wsb = big.tile([_P, _MW * 2 * _D], mybir.dt.bfloat16)
            nc.scalar.dma_start(out=wsb[:], in_=w_d.ap())

            # broadcast rr across partitions: rrb[p, i] = rr[i], fp32 exact
            rrb = psum_rr.tile([_P, _MW * _P], mybir.dt.float32)
            nc.tensor.matmul(rrb[:], ones[:], rrsb[:], start=True, stop=True)

            # one-hot halves: pt0[j, i] = (rr[i] == j), pt1[j, i] = (rr[i] == 128+j)
            pt0 = misc.tile([_P, _MW * _P], mybir.dt.bfloat16)
            nc.vector.tensor_scalar(
                out=pt0[:], in0=rrb[:], scalar1=iota_col[:], scalar2=None,
                op0=mybir.AluOpType.is_equal,
            )
            pt1 = misc.tile([_P, _MW * _P], mybir.dt.bfloat16)
            nc.vector.tensor_scalar(
                out=pt1[0:64, :], in0=rrb[0:64, :], scalar1=float(_P),
                scalar2=iota_col[0:64, :],
                op0=mybir.AluOpType.subtract, op1=mybir.AluOpType.is_equal,
            )

            dist = misc.tile([_P, _M], mybir.dt.float32)

            def rowsum(t, diff, on_dve):
                if not on_dve:
                    sq = work.tile([_P, _D], mybir.dt.bfloat16, tag="sq")
                    nc.scalar.activation(
                        out=sq[:], in_=diff[:],
                        func=mybir.ActivationFunctionType.Square,
                        accum_out=dist[:, t : t + 1],
                    )
                else:
                    sq = work.tile([_P, _D], mybir.dt.bfloat16, tag="sqv")
                    nc.vector.scalar_tensor_tensor(
                        out=sq[:], in0=diff[:], scalar=0.0, in1=diff[:],
                        op0=mybir.AluOpType.bypass, op1=mybir.AluOpType.mult,
                        accum_out=dist[:, t : t + 1],
                    )

            # ---- window chunks first: they fill the idle window while the
            # SWDGE chain generates descriptors, keeping the kernel tail on
            # SWDGE chunk 5 only.
            for k in range(_MW):
                t = _MS + k
                gt = psum_g.tile([_P, _D], mybir.dt.float32, tag="g")
                nc.tensor.matmul(
                    gt[:], pt0[:, k * _P : (k + 1) * _P],
                    wsb[:, (2 * k) * _D : (2 * k + 1) * _D],
                    start=True, stop=False,
                )
                nc.tensor.matmul(
                    gt[:], pt1[0:64, k * _P : (k + 1) * _P],
                    wsb[0:64, (2 * k + 1) * _D : (2 * k + 2) * _D],
                    start=False, stop=True,
                )
                diffw = work.tile([_P, _D], mybir.dt.bfloat16, tag="diffw")
                nc.vector.tensor_tensor(
                    out=diffw[:],
                    in0=xsb[:, t * _D : (t + 1) * _D],
                    in1=gt[:],
                    op=mybir.AluOpType.subtract,
                )
                rowsum(t, diffw, False)  # window squares on ACT (early)

            # ---- SWDGE chunks
            _DVE_SQ = {2, 4}  # chunks whose square+rowsum runs on DVE
            g = big.tile([_P, _MS * _D], mybir.dt.bfloat16)
            g3 = g[:].rearrange("p (m d) -> p m d", d=_D)
            for m in range(_MS):
                h = nc.gpsimd.indirect_dma_start(
                    out=g3[:, m, :],
                    out_offset=None,
                    in_=cen_d.ap(),
                    in_offset=bass.IndirectOffsetOnAxis(
                        ap=idx_sb[:, m : m + 1], axis=0
                    ),
                )
                if m % _N_QUEUES:
                    h.ins.queue = "qPoolDynamic1"
                gather_h.append(h)
                diff = work.tile([_P, _D], mybir.dt.bfloat16, tag="diff")
                nc.vector.tensor_tensor(
                    out=diff[:],
                    in0=xsb[:, m * _D : (m + 1) * _D],
                    in1=g[:, m * _D : (m + 1) * _D],
                    op=mybir.AluOpType.subtract,
                )
                rowsum(m, diff, m in _DVE_SQ)

            # clip both bounds: columns 0-6 early, column 5 (the tail chunk)
            # rides the second tiny op.
            nc.vector.tensor_scalar(
                out=dist[:, : _M - 1],
                in0=dist[:, : _M - 1],
                scalar1=_CLAMP_MIN,
                scalar2=_CLAMP_MAX,
                op0=mybir.AluOpType.max,
                op1=mybir.AluOpType.min,
            )
            nc.vector.tensor_scalar(
                out=dist[:, _M - 1 :],
                in0=dist[:, _M - 1 :],
                scalar1=_CLAMP_MIN,
                scalar2=_CLAMP_MAX,
                op0=mybir.AluOpType.max,
                op1=mybir.AluOpType.min,
            )

            nc.sync.dma_start(out=out_d.ap()[:, :], in_=dist[:])
    gather_h[0].wait_op(idx_sem, 16, "sem-ge")
    nc.compile()
    return nc


def _prep_core(x_bf16_sorted, labels_sorted, centers_bf16, c):
    """Build one core's in_map from the globally sorted arrays."""
    sl = slice(c * _B_LOC, (c + 1) * _B_LOC)
    xs = x_bf16_sorted[sl]
    ls = labels_sorted[sl]

    lab = ls.reshape(_M, _P)  # [t, p]: sample (t, p) = index t*128+p

    wins = np.zeros((_MW, 2, _P, _D), dtype=centers_bf16.dtype)
    rr = np.empty(_MW * _P, dtype=np.float32)
    for k in range(_MW):
        chunk = lab[_MS + k]
        base = min(int(chunk[0]), _C - _W)
        if int(chunk[-1]) - base >= _W:
            return None
        wins[k, 0] = centers_bf16[base : base + _P]
        wins[k, 1, : _W - _P] = centers_bf16[base + _P : base + _W]
        rr[k * _P : (k + 1) * _P] = chunk - base

    return {
        "x": np.ascontiguousarray(
            xs.reshape(_M, _P, _D).transpose(1, 0, 2).reshape(_P, _M * _D)
        ),
        "centers": centers_bf16,
        "labels_packed": np.ascontiguousarray(lab[:_MS].T.astype(np.int32)),
        "wins": np.ascontiguousarray(
            wins.transpose(2, 0, 1, 3).reshape(_P, -1)
        ),
        "rr": np.ascontiguousarray(
            rr.astype(ml_dtypes.bfloat16).reshape(1, -1)
        ),
    }


def _run(x, labels, centers, trace=False, **hw_kwargs):
    from concourse import bass_utils

    if "nc" not in _cache:
        _cache["nc"] = _build()
    nc = _cache["nc"]

    x = np.asarray(x, dtype=np.float32).astype(ml_dtypes.bfloat16)
    labels = np.asarray(labels).astype(np.int64)
    centers = np.ascontiguousarray(
        np.asarray(centers, dtype=np.float32).astype(ml_dtypes.bfloat16)
    )
    assert x.shape == (_B, _D) and labels.shape == (_B,) and centers.shape == (_C, _D)
    assert labels.min() >= 0 and labels.max() < _C

    order = np.argsort(labels, kind="stable")
    x_sorted = x[order]
    labels_sorted = labels[order]

    in_maps = []
    for c in range(_N_CORES):
        m = _prep_core(x_sorted, labels_sorted, centers, c)
        if m is None:
            raise RuntimeError("window overflow — SWDGE fallback required")
        in_maps.append(m)

    r = bass_utils.run_bass_kernel_spmd(
        nc, in_maps, core_ids=list(range(_N_CORES)), trace=trace, **hw_kwargs
    )
    total = sum(res["out"].astype(np.float64).sum() for res in r.results)
    return np.array(total / _B, dtype=np.float32), r


def kernel(x, labels, centers):
    out, _ = _run(x, labels, centers, trace=False)
    return out
